# revision 88
# baseline (speedup 1.0000x reference)
"""BagRE segment-mean + classifier kernel for 8 Trainium2 NeuronCores.

Problem:  hidden [262144, 256] f32, sorted bag_id [262144] i64 with 8192 bags,
          W [128, 256], b [128]  ->  logits [8192, 128] f32
          logits = (segment_mean(hidden, bag_id) @ W.T) + b

Strategy (v2 — fp8e4 DoubleRow):
  bag_id is sorted -> rows per bag are contiguous.  Core k owns bags
  [1024k, 1024(k+1)), split into 8 blocks of 128 bags.  Every bag is padded
  host-side to an EVEN number of rows (zero rows add nothing to the sum), so
  consecutive row pairs (2q, 2q+1) always share a bag.  The padded stream is
  packed in 256-row "double tiles": partition p holds rows (2p, 2p+1) as the
  two K-subtiles of a DoubleRow fp8e4 matmul.  One [128, 128] fp8 one-hot
  per double tile (broadcast over the K-pair dim with a stride-0 AP) then
  reduces 256 rows per matmul at 0.5 cycles/column — 2x the fp16 PE pace —
  and halves the DVE one-hot work vs a per-128-row-tile scheme.

  hidden is quantized host-side to fp8 E4M3 with error feedback down each
  (bag, h) column, so the bag-sum error telescopes to one quantum.  The
  whole fp8 stream (~74KB/partition) is SBUF-resident: every chunk DMA is
  issued up front on the sync/scalar HWDGE rings (alternating, small chunks
  first while the DMA ramps), so no transfer ever waits on compute.  Consts
  ride the gpsimd ring; small transfers crawl and would stall data queued
  behind them.

  One-hots come from DVE is_equal+mult writing uint16 words whose bytes are
  the fp8 encodings (fp8 writes from the DVE are ~7x slower), with the head
  and every HOSTMOD-th tile precomputed on the host inside the stream
  payload.  Warmup matmuls hold the PE clock up through the DMA ramp, and
  tiny filler matmuls pace the PE just under the DMA supply rate so it
  never pays the ~0.9us semaphore round trip of catching the stream.

  Finalize is a 3-stage pipeline, each stage one block behind the stream:
  block j's PSUM sums are copied to SBUF f16 (ACT) at block j's end;
  PE-transposed to [h, bags] f16 at block j+1's end; classifier GEMM +
  fused recip/bias + f16 store at block j+2's end.
"""

import os
import sys
import bisect
import contextlib
import numpy as np

try:
    import concourse.bass as bass  # noqa: F401
except Exception:  # pragma: no cover
    sys.path.insert(0, "/opt/trn_rl_repo")

import concourse.bass as bass
import concourse.tile as tile
from concourse import mybir, bacc, masks
from concourse.bass_utils import run_bass_kernel_spmd

F8E4 = mybir.dt.np(mybir.dt.float8e4)

N = 262144
H = 256
C = 128
NUM_BAGS = 8192
NCORES = 8
BLOCK_BAGS = 128
BLOCKS_PER_CORE = NUM_BAGS // BLOCK_BAGS // NCORES   # 8
ROWS_DT = 256                                        # rows per double tile

CH2 = int(os.environ.get("BK_CH2", "24"))            # steady-state chunk size
FC2 = int(os.environ.get("BK_FC2", "6"))             # first two (short) chunks
MC2 = int(os.environ.get("BK_MC2", "12"))            # third (medium) chunk
HOSTMOD = int(os.environ.get("BK_HOSTMOD", "4"))     # every Nth one-hot from host
HEAD_OH = int(os.environ.get("BK_HEADOH", "12"))     # host one-hots up front
WARMUP_MM = int(os.environ.get("BK_WARMUP", "12"))
# PE pacing fillers: the PE consumes 587 B/ns of stream at full speed but
# the DMA supplies ~420 (less during the ramp); zero-tile filler matmuls
# keep the PE just below DMA pace so it never stalls on a segment sem
# (each stall costs ~1us and risks a HAM duty downgrade)
FILL_N = int(os.environ.get("BK_FN", "1"))           # fillers per dt
FILL_START = int(os.environ.get("BK_FSTART", "32"))  # none during DMA ramp
FILL_END = int(os.environ.get("BK_FEND", "120"))     # none after (DMA done)
A_BUFS = int(os.environ.get("BK_ABUFS", "64"))
# 0 = one buffer per chunk: the whole fp8 stream (~74KB/partition) lives in
# SBUF, so chunk DMAs never wait on PE progress
H_BUFS = int(os.environ.get("BK_HBUFS", "0"))


def _is_host(t):
    if t < HEAD_OH:
        return True
    return HOSTMOD and t % HOSTMOD == HOSTMOD - 1


def _chunk_layout(T2):
    """Chunks of double tiles; each chunk's DMA payload is its hid bytes
    followed by its host one-hot tiles (merged so one dma_start per chunk
    keeps the ring queues short).  Small chunks up front for a fast start,
    large ones after (each chunk boundary the PE crosses before the DMA
    costs ~0.9us of semaphore latency, so fewer boundaries win).
    Returns (chunks, offsets, widths)."""
    # gradual ladder: per-queue DMA rate ramps over the first ~8us, so early
    # chunks must stay small for the PE to cross boundaries without waiting
    sizes = [FC2, FC2, 8, 8, MC2, MC2, 16]
    chunks = []
    pos = 0
    i = 0
    while pos < T2:
        step = sizes[i] if i < len(sizes) else CH2
        i += 1
        chunks.append((pos, min(pos + step, T2)))
        pos = chunks[-1][1]
    offs = [0]
    widths = []
    for t0, t1 in chunks:
        nh = sum(1 for t in range(t0, t1) if _is_host(t))
        w = (t1 - t0) * 2 * H + nh * 128
        widths.append(w)
        offs.append(offs[-1] + w)
    return chunks, offs, widths

LAST_RESULTS = None
_prog_cache = {}


def _install_ntff_shim():
    """Register the axon NTFF profiling hook so trace=True works."""
    try:
        from antenv.axon_hooks import get_axon_ntff_profile_hook  # noqa: F401
        return True
    except Exception:
        pass
    try:
        import types
        import antenv
        from trn_agent_boot.trn_boot import _ntff_profile_via_ctypes

        hook = _ntff_profile_via_ctypes("/opt/axon/libaxon_pjrt.so")
        if hook is None:
            return False
        mod = types.ModuleType("antenv.axon_hooks")
        mod._hook = hook
        mod.get_axon_ntff_profile_hook = lambda: mod._hook
        mod.set_axon_ntff_profile_hook = lambda h: setattr(mod, "_hook", h)
        sys.modules["antenv.axon_hooks"] = mod
        antenv.axon_hooks = mod
        import concourse.bass_utils as bu

        orig_upload = bu.upload_artifacts

        def _safe_upload(tmpdir):
            try:
                return orig_upload(tmpdir)
            except Exception:
                return tmpdir

        bu.upload_artifacts = _safe_upload
        return True
    except Exception:
        return False


def _build_program(pos_tblks):
    T2 = sum(pos_tblks)
    offs = [0]
    for tb in pos_tblks:
        offs.append(offs[-1] + tb)
    chunks, coffs, cwidths = _chunk_layout(T2)

    f32 = mybir.dt.float32
    f16 = mybir.dt.float16
    f8e4 = mybir.dt.float8e4
    DR = mybir.MatmulPerfMode.DoubleRow

    u16 = mybir.dt.uint16
    # consts live at the tail of the hid stream tensor and ride the scalar
    # data ring (the gpsimd SWDGE ring lands them ~3us too late for the
    # first DVE one-hot): [relh (T2) | vsel (T2) | f16 consts as f32 pairs]
    # f16 part (bitcast): [b (C) | recip (8) | iota (128)]
    CW = C + BLOCKS_PER_CORE + 128
    CST_W = (2 * T2 + CW // 2) * 4          # bytes per partition
    nc = bacc.Bacc(trn_type="TRN2", target_bir_lowering=False, debug=False)
    hid = nc.dram_tensor("hid", [128, coffs[-1] + CST_W], f8e4,
                         kind="ExternalInput").ap()
    wt = nc.dram_tensor("wt", [128, 2 * C], f16, kind="ExternalInput").ap()
    out = nc.dram_tensor("out", [BLOCKS_PER_CORE, 128, C], f16,
                         kind="ExternalOutput").ap()

    with tile.TileContext(nc) as tc:
        with contextlib.ExitStack() as ctx:
            consts = ctx.enter_context(tc.tile_pool(name="consts", bufs=1))
            a_pool = ctx.enter_context(
                tc.tile_pool(name="onehot", bufs=A_BUFS))
            sums_pool = ctx.enter_context(tc.tile_pool(name="sums", bufs=4))
            sT_pool = ctx.enter_context(tc.tile_pool(name="sT", bufs=6))
            ob_pool = ctx.enter_context(tc.tile_pool(name="ob", bufs=3))
            psum_s = ctx.enter_context(
                tc.tile_pool(name="psum_s", bufs=3, space="PSUM"))
            psum_t = ctx.enter_context(
                tc.tile_pool(name="psum_t", bufs=2, space="PSUM"))
            psum_o = ctx.enter_context(
                tc.tile_pool(name="psum_o", bufs=2, space="PSUM"))
            psum_w = ctx.enter_context(
                tc.tile_pool(name="psum_w", bufs=1, space="PSUM"))

            # --- PE warmup: hold the PE clock up while DMA ramps ---------
            wz = consts.tile([128, 2, H], f8e4)
            nc.vector.memset(wz[:], 1.0)
            warm = psum_w.tile([128, H], f32)
            for i in range(WARMUP_MM):
                nc.tensor.matmul(warm[:], wz[:, :, 0:128], wz[:],
                                 start=(i == 0), stop=(i == WARMUP_MM - 1),
                                 perf_mode=DR)

            def warm_fill():
                # tiny filler (~35ns incl. its 64-elem ldweights): paces the
                # PE to just under the DMA supply rate so it never pays the
                # ~0.9us semaphore-propagation cost of catching the stream,
                # and keeps the HAM activity up
                nc.tensor.matmul(warm[0:32, 0:64], wz[:, :, 0:32],
                                 wz[:, :, 0:64],
                                 start=True, stop=True, perf_mode=DR)

            # consts: cst16 (rel/b/recip/iota) first on the scalar HWDGE
            # ring (fast; needed by the very first one-hot), wt on gpsimd
            # (first needed by the classifier, several us in)
            wt_t = consts.tile([128, 2 * C], f16)
            nc.gpsimd.dma_start(wt_t[:], wt[:])
            ident_t = consts.tile([128, 128], f16)
            masks.make_identity(nc, ident_t[:])

            pend_t = []    # blocks awaiting PE transpose
            pend_c = []    # blocks awaiting classifier

            def stage_t(item):
                j, sums_t = item
                sT = []
                for q in range(2):
                    p_t = psum_t.tile([128, 128], f16, tag="psum_t")
                    nc.tensor.transpose(
                        p_t[:], sums_t[:, q * 128:(q + 1) * 128], ident_t[:])
                    s_t = sT_pool.tile([128, 128], f16, tag="sT")
                    nc.scalar.copy(s_t[:], p_t[:])
                    sT.append(s_t)
                pend_c.append((j, sT[0], sT[1]))

            def stage_c(item):
                j, sT0, sT1 = item
                po = psum_o.tile([128, C], f32, tag="po")
                nc.tensor.matmul(po[:], sT0[:], wt_t[:, 0:C],
                                 start=True, stop=False)
                nc.tensor.matmul(po[:], sT1[:], wt_t[:, C:2 * C],
                                 start=False, stop=True)
                ob = ob_pool.tile([128, C], f16, tag="ob")
                nc.vector.scalar_tensor_tensor(
                    ob[:], po[:], recip_t[:, j:j + 1], b_t,
                    mybir.AluOpType.mult, mybir.AluOpType.add)
                nc.sync.dma_start(out[j], ob[:])

            # issue ALL chunk DMAs up front: the stream fits in SBUF, and
            # keeping the ring queues trigger-only means no chunk ever waits
            # behind a finalize ACTIVATE that is itself gated on PE progress
            # One SBUF-resident tile holds the whole fp8 stream (~74KB per
            # partition); chunk segments alternate the sync/scalar HWDGE
            # rings (7 triggers each) so data arrives in consumption order
            # at the combined two-queue rate (a single HWDGE queue tops out
            # ~330 B/ns; the PE eats 587).  All triggers are issued up
            # front — nothing ever gates the DMA on compute progress.
            stream_t = consts.tile([128, coffs[-1] + CST_W], f8e4)
            cst_all = stream_t[:, coffs[-1]:coffs[-1] + CST_W].bitcast(f32)
            relT = cst_all[:, 0:T2]
            vselT = cst_all[:, T2:2 * T2]
            cst_t = cst_all[:, 2 * T2:2 * T2 + CW // 2].bitcast(f16)
            b_t = cst_t[:, 0:C]
            recip_t = cst_t[:, C:C + BLOCKS_PER_CORE]
            iota_t = cst_t[:, C + BLOCKS_PER_CORE:CW]
            for c, (t0, t1) in enumerate(chunks):
                W = cwidths[c]
                o0 = coffs[c]
                dma_eng = nc.sync if (c % 2 == 0) else nc.scalar
                if c == 0:
                    h0 = ((t1 - t0) // 2) * 2 * H
                    nc.sync.dma_start(stream_t[:, o0:o0 + h0],
                                      hid[:, o0:o0 + h0])
                    nc.scalar.dma_start(stream_t[:, o0 + h0:o0 + W],
                                        hid[:, o0 + h0:o0 + W])
                    # consts right behind chunk0a on the sync ring: they
                    # land ~10.5us, ahead of the first DVE one-hot (dt 12)
                    nc.sync.dma_start(
                        stream_t[:, coffs[-1]:coffs[-1] + CST_W],
                        hid[:, coffs[-1]:coffs[-1] + CST_W])
                else:
                    dma_eng.dma_start(stream_t[:, o0:o0 + W],
                                      hid[:, o0:o0 + W])

            psum_cur = None
            for c, (t0, t1) in enumerate(chunks):
                L = t1 - t0
                hid_t = stream_t[:, coffs[c]:coffs[c] + cwidths[c]]
                oh_base = L * 2 * H
                n_sl = 0

                for t in range(t0, t1):
                    j = bisect.bisect_right(offs, t) - 1
                    i = t - offs[j]
                    tb = pos_tblks[j]

                    if _is_host(t):
                        sl = n_sl
                        n_sl += 1
                        oh_ap = hid_t[:, oh_base + sl * 128:
                                      oh_base + (sl + 1) * 128]
                    else:
                        # DoubleRow one-hot via uint16 bit tricks: bags are
                        # even-padded so rows (2p, 2p+1) share rel.  The fp8
                        # one-hot row is 64 uint16 words with word (rel//2)
                        # equal to 0x0038 (rel even) or 0x3800 (rel odd) —
                        # fp8e4(1.0) in the right byte.  (ihalf == rel//2)
                        # * vsel computes it with all-16-bit tensor operands
                        # (fp8 writes from the DVE measure ~7x slower).
                        a_t = a_pool.tile([128, 64], u16, tag="onehot")
                        nc.vector.tensor_scalar(
                            a_t[:], iota_t[:, 0:64], relT[:, t:t + 1],
                            vselT[:, t:t + 1],
                            mybir.AluOpType.is_equal, mybir.AluOpType.mult)
                        oh_ap = a_t[:].bitcast(f8e4)
                    # the pair rows share one one-hot: broadcast it over the
                    # K-pair dim (stride-0 AP)
                    lhsT = oh_ap.unsqueeze(1).broadcast_to([128, 2, 128])

                    rhs = hid_t[:, (t - t0) * 2 * H:(t - t0 + 1) * 2 * H] \
                        .rearrange("p (two h) -> p two h", two=2)
                    if i == 0:
                        psum_cur = psum_s.tile([128, H], f32, tag="psum_s")
                    nc.tensor.matmul(
                        psum_cur[:], lhsT, rhs,
                        start=(i == 0), stop=(i == tb - 1), perf_mode=DR)
                    if FILL_START <= t < FILL_END:
                        for _ in range(FILL_N):
                            warm_fill()

                    if i == tb - 1:
                        sums_t = sums_pool.tile([128, H], f16, tag="sums")
                        nc.scalar.copy(sums_t[:], psum_cur[:])
                        pend_t.append((j, sums_t))
                        if len(pend_t) > 1:
                            stage_t(pend_t.pop(0))
                        if len(pend_c) > 1:
                            stage_c(pend_c.pop(0))
            while pend_t:
                stage_t(pend_t.pop(0))
            while pend_c:
                stage_c(pend_c.pop(0))
    nc.compile()
    return nc


def _quantize_ef(hidden, bag_edges):
    """fp8 E4M3 with per-(bag, h) error feedback down the rows."""
    starts = bag_edges[:-1]
    lens = np.diff(bag_edges)
    hq = np.zeros((N, H), F8E4)
    carry = np.zeros((NUM_BAGS, H), np.float32)
    for k in range(int(lens.max())):
        m = lens > k
        idx = starts[m] + k
        v = hidden[idx] + carry[m]
        q = v.astype(F8E4)
        hq[idx] = q
        carry[m] = v - q.astype(np.float32)
    return hq


def _pack_inputs(hidden, W, b, bag_id):
    counts = np.bincount(bag_id, minlength=NUM_BAGS)
    recip_all = (1.0 / np.maximum(counts, 1)).astype(np.float32)

    bag_edges = np.searchsorted(bag_id, np.arange(NUM_BAGS + 1))
    hq = _quantize_ef(hidden, bag_edges)

    lens = np.diff(bag_edges)                       # [8192]
    plens = lens + (lens & 1)                       # even-padded
    nblocks = NUM_BAGS // BLOCK_BAGS                # 64
    blk_plen = plens.reshape(nblocks, BLOCK_BAGS).sum(axis=1)
    tiles2 = np.maximum(1, -(-blk_plen // ROWS_DT))
    pos_tblks = tuple(
        int(x) for x in
        tiles2.reshape(NCORES, BLOCKS_PER_CORE).max(axis=0))
    T2 = sum(pos_tblks)
    offs2 = np.concatenate([[0], np.cumsum(pos_tblks)])

    Xp = np.zeros((NCORES, T2 * ROWS_DT, H), F8E4)
    rel2 = np.full((NCORES, T2 * 128), -1.0, dtype=np.float16)
    for bidx in range(nblocks):
        k, j = divmod(bidx, BLOCKS_PER_CORE)
        bl = lens[bidx * BLOCK_BAGS:(bidx + 1) * BLOCK_BAGS]
        pl = plens[bidx * BLOCK_BAGS:(bidx + 1) * BLOCK_BAGS]
        starts_dst = (offs2[j] * ROWS_DT +
                      np.concatenate([[0], np.cumsum(pl)[:-1]]))
        for bi in range(BLOCK_BAGS):
            Lb = int(bl[bi])
            d = int(starts_dst[bi])
            if Lb:
                s = int(bag_edges[bidx * BLOCK_BAGS + bi])
                Xp[k, d:d + Lb] = hq[s:s + Lb]
            PLb = int(pl[bi])
            if PLb:
                rel2[k, d // 2:(d + PLb) // 2] = bi

    wt_np = np.ascontiguousarray(W.T).astype(np.float16)      # [256, 128]
    wt_packed = np.concatenate([wt_np[0:128], wt_np[128:256]],
                               axis=1)                        # [128, 2C] f16
    b_np = np.tile(b.astype(np.float16), (128, 1))
    iota_np = np.tile((np.arange(128) % 64).astype(np.float16), (128, 1))

    chunks, coffs, cwidths = _chunk_layout(T2)
    in_maps = []
    for k in range(NCORES):
        hidc = (Xp[k].reshape(T2, 128, 2, H).transpose(1, 0, 2, 3)
                .reshape(128, T2 * 2 * H))
        rk = rel2[k].reshape(T2, 128)
        stream = np.zeros((128, coffs[-1]), F8E4)
        for c, (t0, t1) in enumerate(chunks):
            o0 = coffs[c]
            hw_ = (t1 - t0) * 2 * H
            stream[:, o0:o0 + hw_] = hidc[:, t0 * 2 * H:t1 * 2 * H]
            sl = 0
            for t in range(t0, t1):
                if _is_host(t):
                    rr = rk[t].astype(np.int32)
                    valid = rr >= 0
                    oh = np.zeros((128, 128), F8E4)
                    oh[np.arange(128)[valid], rr[valid]] = 1.0
                    o = o0 + hw_ + sl * 128
                    stream[:, o:o + 128] = oh
                    sl += 1
        relc = rel2[k].reshape(T2, 128).T.astype(np.float32)  # [128, T2]
        relh = np.where(relc >= 0, np.floor(relc / 2), -1.0)
        vsel = np.where(relc.astype(np.int32) % 2 == 0, 56.0, 14336.0)
        recc = recip_all[k * 1024:(k + 1) * 1024].reshape(
            BLOCKS_PER_CORE, 128).T.astype(np.float16)        # [128, 8]
        cst16_np = np.concatenate(
            [b_np, recc, iota_np], axis=1).astype(np.float16)
        cst_np = np.concatenate(
            [relh.astype(np.float32), vsel.astype(np.float32),
             np.ascontiguousarray(cst16_np).view(np.float32)], axis=1)
        cst_bytes = np.ascontiguousarray(cst_np).view(np.uint8).view(F8E4)
        in_maps.append({
            "hid": np.ascontiguousarray(
                np.concatenate([stream, cst_bytes], axis=1)),
            "wt": np.ascontiguousarray(wt_packed),
        })
    return in_maps, pos_tblks


def kernel(hidden, W, b, bag_id):
    global LAST_RESULTS
    hidden = np.asarray(hidden, dtype=np.float32)
    W = np.asarray(W, dtype=np.float32)
    b = np.asarray(b, dtype=np.float32)
    bag_id = np.asarray(bag_id).astype(np.int64)

    in_maps, pos_tblks = _pack_inputs(hidden, W, b, bag_id)

    key = (pos_tblks, CH2, FC2, MC2, HOSTMOD, HEAD_OH, WARMUP_MM, FILL_N,
           FILL_START, FILL_END, A_BUFS, H_BUFS)
    if key not in _prog_cache:
        _prog_cache[key] = _build_program(pos_tblks)
    nc = _prog_cache[key]

    trace = False
    if os.environ.get("BASS_TRACE"):
        trace = _install_ntff_shim()

    res = run_bass_kernel_spmd(nc, in_maps, core_ids=list(range(NCORES)),
                               trace=trace)
    LAST_RESULTS = res

    out = np.concatenate(
        [np.asarray(res.results[k]["out"]).astype(np.float32).reshape(1024, C)
         for k in range(NCORES)], axis=0)
    return out


# revision 92
# speedup vs baseline: 1.0404x; 1.0404x over previous
"""BagRE segment-mean + classifier kernel for 8 Trainium2 NeuronCores.

Problem:  hidden [262144, 256] f32, sorted bag_id [262144] i64 with 8192 bags,
          W [128, 256], b [128]  ->  logits [8192, 128] f32
          logits = (segment_mean(hidden, bag_id) @ W.T) + b

Strategy (v2 — fp8e4 DoubleRow):
  bag_id is sorted -> rows per bag are contiguous.  Core k owns bags
  [1024k, 1024(k+1)), split into 8 blocks of 128 bags.  Every bag is padded
  host-side to an EVEN number of rows (zero rows add nothing to the sum), so
  consecutive row pairs (2q, 2q+1) always share a bag.  The padded stream is
  packed in 256-row "double tiles": partition p holds rows (2p, 2p+1) as the
  two K-subtiles of a DoubleRow fp8e4 matmul.  One [128, 128] fp8 one-hot
  per double tile (broadcast over the K-pair dim with a stride-0 AP) then
  reduces 256 rows per matmul at 0.5 cycles/column — 2x the fp16 PE pace —
  and halves the DVE one-hot work vs a per-128-row-tile scheme.

  hidden is quantized host-side to fp8 E4M3 with error feedback down each
  (bag, h) column, so the bag-sum error telescopes to one quantum.  The
  whole fp8 stream (~74KB/partition) is SBUF-resident: every chunk DMA is
  issued up front on the sync/scalar HWDGE rings (alternating, small chunks
  first while the DMA ramps), so no transfer ever waits on compute.  Consts
  ride the gpsimd ring; small transfers crawl and would stall data queued
  behind them.

  One-hots come from DVE is_equal+mult writing uint16 words whose bytes are
  the fp8 encodings (fp8 writes from the DVE are ~7x slower), with the head
  and every HOSTMOD-th tile precomputed on the host inside the stream
  payload.  Warmup matmuls hold the PE clock up through the DMA ramp, and
  tiny filler matmuls pace the PE just under the DMA supply rate so it
  never pays the ~0.9us semaphore round trip of catching the stream.

  Finalize is a 3-stage pipeline, each stage one block behind the stream:
  block j's PSUM sums are copied to SBUF f16 (ACT) at block j's end;
  PE-transposed to [h, bags] f16 at block j+1's end; classifier GEMM +
  fused recip/bias + f16 store at block j+2's end.
"""

import os
import sys
import bisect
import contextlib
import numpy as np

try:
    import concourse.bass as bass  # noqa: F401
except Exception:  # pragma: no cover
    sys.path.insert(0, "/opt/trn_rl_repo")

import concourse.bass as bass
import concourse.tile as tile
from concourse import mybir, bacc, masks
from concourse.bass_utils import run_bass_kernel_spmd

F8E4 = mybir.dt.np(mybir.dt.float8e4)

N = 262144
H = 256
C = 128
NUM_BAGS = 8192
NCORES = 8
BLOCK_BAGS = 128
BLOCKS_PER_CORE = NUM_BAGS // BLOCK_BAGS // NCORES   # 8
ROWS_DT = 256                                        # rows per double tile

CH2 = int(os.environ.get("BK_CH2", "24"))            # steady-state chunk size
FC2 = int(os.environ.get("BK_FC2", "6"))             # first two (short) chunks
MC2 = int(os.environ.get("BK_MC2", "12"))            # third (medium) chunk
HOSTMOD = int(os.environ.get("BK_HOSTMOD", "4"))     # every Nth one-hot from host
HEAD_OH = int(os.environ.get("BK_HEADOH", "12"))     # host one-hots up front
WARMUP_MM = int(os.environ.get("BK_WARMUP", "12"))
# PE pacing fillers: the PE consumes 587 B/ns of stream at full speed but
# the DMA supplies ~420 (less during the ramp); zero-tile filler matmuls
# keep the PE just below DMA pace so it never stalls on a segment sem
# (each stall costs ~1us and risks a HAM duty downgrade)
FILL_N = int(os.environ.get("BK_FN", "1"))           # fillers per dt
FILL_START = int(os.environ.get("BK_FSTART", "32"))  # none during DMA ramp
FILL_END = int(os.environ.get("BK_FEND", "120"))     # none after (DMA done)
A_BUFS = int(os.environ.get("BK_ABUFS", "64"))
# 0 = one buffer per chunk: the whole fp8 stream (~74KB/partition) lives in
# SBUF, so chunk DMAs never wait on PE progress
H_BUFS = int(os.environ.get("BK_HBUFS", "0"))


def _is_host(t):
    if t < HEAD_OH:
        return True
    return HOSTMOD and t % HOSTMOD == HOSTMOD - 1


def _chunk_layout(T2):
    """Chunks of double tiles; each chunk's DMA payload is its hid bytes
    followed by its host one-hot tiles (merged so one dma_start per chunk
    keeps the ring queues short).  Small chunks up front for a fast start,
    large ones after (each chunk boundary the PE crosses before the DMA
    costs ~0.9us of semaphore latency, so fewer boundaries win).
    Returns (chunks, offsets, widths)."""
    # gradual ladder: per-queue DMA rate ramps over the first ~8us, so early
    # chunks must stay small for the PE to cross boundaries without waiting
    sizes = [FC2, FC2, 8, 8, MC2, MC2, 16]
    chunks = []
    pos = 0
    i = 0
    while pos < T2:
        step = sizes[i] if i < len(sizes) else CH2
        i += 1
        chunks.append((pos, min(pos + step, T2)))
        pos = chunks[-1][1]
    offs = [0]
    widths = []
    for t0, t1 in chunks:
        nh = sum(1 for t in range(t0, t1) if _is_host(t))
        w = (t1 - t0) * 2 * H + nh * 128
        widths.append(w)
        offs.append(offs[-1] + w)
    return chunks, offs, widths

LAST_RESULTS = None
_prog_cache = {}


def _install_ntff_shim():
    """Register the axon NTFF profiling hook so trace=True works."""
    try:
        from antenv.axon_hooks import get_axon_ntff_profile_hook  # noqa: F401
        return True
    except Exception:
        pass
    try:
        import types
        import antenv
        from trn_agent_boot.trn_boot import _ntff_profile_via_ctypes

        hook = _ntff_profile_via_ctypes("/opt/axon/libaxon_pjrt.so")
        if hook is None:
            return False
        mod = types.ModuleType("antenv.axon_hooks")
        mod._hook = hook
        mod.get_axon_ntff_profile_hook = lambda: mod._hook
        mod.set_axon_ntff_profile_hook = lambda h: setattr(mod, "_hook", h)
        sys.modules["antenv.axon_hooks"] = mod
        antenv.axon_hooks = mod
        import concourse.bass_utils as bu

        orig_upload = bu.upload_artifacts

        def _safe_upload(tmpdir):
            try:
                return orig_upload(tmpdir)
            except Exception:
                return tmpdir

        bu.upload_artifacts = _safe_upload
        return True
    except Exception:
        return False


def _build_program(pos_tblks):
    T2 = sum(pos_tblks)
    offs = [0]
    for tb in pos_tblks:
        offs.append(offs[-1] + tb)
    chunks, coffs, cwidths = _chunk_layout(T2)

    f32 = mybir.dt.float32
    f16 = mybir.dt.float16
    f8e4 = mybir.dt.float8e4
    DR = mybir.MatmulPerfMode.DoubleRow

    u16 = mybir.dt.uint16
    # cst: [relh (T2) | vsel (T2) | f16 consts as f32 pairs]
    # f16 part (bitcast): [b (C) | recip (8) | iota (128)]
    CW = C + BLOCKS_PER_CORE + 128
    nc = bacc.Bacc(trn_type="TRN2", target_bir_lowering=False, debug=False)
    hid = nc.dram_tensor("hid", [128, coffs[-1]], f8e4,
                         kind="ExternalInput").ap()
    cst = nc.dram_tensor("cst", [128, 2 * T2 + CW // 2], f32,
                         kind="ExternalInput").ap()
    wt = nc.dram_tensor("wt", [128, 2 * C], f16, kind="ExternalInput").ap()
    out = nc.dram_tensor("out", [BLOCKS_PER_CORE, 128, C], f16,
                         kind="ExternalOutput").ap()

    with tile.TileContext(nc) as tc:
        with contextlib.ExitStack() as ctx:
            consts = ctx.enter_context(tc.tile_pool(name="consts", bufs=1))
            a_pool = ctx.enter_context(
                tc.tile_pool(name="onehot", bufs=A_BUFS))
            sums_pool = ctx.enter_context(tc.tile_pool(name="sums", bufs=4))
            sT_pool = ctx.enter_context(tc.tile_pool(name="sT", bufs=6))
            ob_pool = ctx.enter_context(tc.tile_pool(name="ob", bufs=3))
            psum_s = ctx.enter_context(
                tc.tile_pool(name="psum_s", bufs=3, space="PSUM"))
            psum_t = ctx.enter_context(
                tc.tile_pool(name="psum_t", bufs=2, space="PSUM"))
            psum_o = ctx.enter_context(
                tc.tile_pool(name="psum_o", bufs=2, space="PSUM"))
            psum_w = ctx.enter_context(
                tc.tile_pool(name="psum_w", bufs=1, space="PSUM"))

            # --- PE warmup: hold the PE clock up while DMA ramps ---------
            wz = consts.tile([128, 2, H], f8e4)
            nc.vector.memset(wz[:], 1.0)
            warm = psum_w.tile([128, H], f32)
            for i in range(WARMUP_MM):
                nc.tensor.matmul(warm[:], wz[:, :, 0:128], wz[:],
                                 start=(i == 0), stop=(i == WARMUP_MM - 1),
                                 perf_mode=DR)

            def warm_fill():
                # tiny filler (~35ns incl. its 64-elem ldweights): paces the
                # PE to just under the DMA supply rate so it never pays the
                # ~0.9us semaphore-propagation cost of catching the stream,
                # and keeps the HAM activity up
                nc.tensor.matmul(warm[0:32, 0:64], wz[:, :, 0:32],
                                 wz[:, :, 0:64],
                                 start=True, stop=True, perf_mode=DR)

            # consts: cst16 (rel/b/recip/iota) first on the scalar HWDGE
            # ring (fast; needed by the very first one-hot), wt on gpsimd
            # (first needed by the classifier, several us in)
            # consts ride the gpsimd ring: small transfers crawl (~20-50
            # B/ns) and would delay the data chunks queued behind them on
            # a data ring
            cst_all = consts.tile([128, 2 * T2 + CW // 2], f32)
            nc.gpsimd.dma_start(cst_all[:], cst[:])
            wt_t = consts.tile([128, 2 * C], f16)
            nc.gpsimd.dma_start(wt_t[:], wt[:])
            relT = cst_all[:, 0:T2]
            vselT = cst_all[:, T2:2 * T2]
            cst_t = cst_all[:, 2 * T2:2 * T2 + CW // 2].bitcast(f16)
            b_t = cst_t[:, 0:C]
            recip_t = cst_t[:, C:C + BLOCKS_PER_CORE]
            iota_t = cst_t[:, C + BLOCKS_PER_CORE:CW]
            ident_t = consts.tile([128, 128], f16)
            masks.make_identity(nc, ident_t[:])

            pend_t = []    # blocks awaiting PE transpose
            pend_c = []    # blocks awaiting classifier

            def stage_t(item):
                j, sums_t = item
                sT = []
                for q in range(2):
                    p_t = psum_t.tile([128, 128], f16, tag="psum_t")
                    nc.tensor.transpose(
                        p_t[:], sums_t[:, q * 128:(q + 1) * 128], ident_t[:])
                    s_t = sT_pool.tile([128, 128], f16, tag="sT")
                    nc.scalar.copy(s_t[:], p_t[:])
                    sT.append(s_t)
                pend_c.append((j, sT[0], sT[1]))

            def stage_c(item):
                j, sT0, sT1 = item
                po = psum_o.tile([128, C], f32, tag="po")
                nc.tensor.matmul(po[:], sT0[:], wt_t[:, 0:C],
                                 start=True, stop=False)
                nc.tensor.matmul(po[:], sT1[:], wt_t[:, C:2 * C],
                                 start=False, stop=True)
                ob = ob_pool.tile([128, C], f16, tag="ob")
                nc.vector.scalar_tensor_tensor(
                    ob[:], po[:], recip_t[:, j:j + 1], b_t,
                    mybir.AluOpType.mult, mybir.AluOpType.add)
                nc.sync.dma_start(out[j], ob[:])

            # issue ALL chunk DMAs up front: the stream fits in SBUF, and
            # keeping the ring queues trigger-only means no chunk ever waits
            # behind a finalize ACTIVATE that is itself gated on PE progress
            # One SBUF-resident tile holds the whole fp8 stream (~74KB per
            # partition); chunk segments alternate the sync/scalar HWDGE
            # rings (7 triggers each) so data arrives in consumption order
            # at the combined two-queue rate (a single HWDGE queue tops out
            # ~330 B/ns; the PE eats 587).  All triggers are issued up
            # front — nothing ever gates the DMA on compute progress.
            stream_t = consts.tile([128, coffs[-1]], f8e4)
            for c, (t0, t1) in enumerate(chunks):
                W = cwidths[c]
                o0 = coffs[c]
                dma_eng = nc.sync if (c % 2 == 0) else nc.scalar
                if c == 0:
                    h0 = ((t1 - t0) // 2) * 2 * H
                    nc.sync.dma_start(stream_t[:, o0:o0 + h0],
                                      hid[:, o0:o0 + h0])
                    nc.scalar.dma_start(stream_t[:, o0 + h0:o0 + W],
                                        hid[:, o0 + h0:o0 + W])
                else:
                    dma_eng.dma_start(stream_t[:, o0:o0 + W],
                                      hid[:, o0:o0 + W])

            psum_cur = None
            for c, (t0, t1) in enumerate(chunks):
                L = t1 - t0
                hid_t = stream_t[:, coffs[c]:coffs[c] + cwidths[c]]
                oh_base = L * 2 * H
                n_sl = 0

                for t in range(t0, t1):
                    j = bisect.bisect_right(offs, t) - 1
                    i = t - offs[j]
                    tb = pos_tblks[j]

                    if _is_host(t):
                        sl = n_sl
                        n_sl += 1
                        oh_ap = hid_t[:, oh_base + sl * 128:
                                      oh_base + (sl + 1) * 128]
                    else:
                        # DoubleRow one-hot via uint16 bit tricks: bags are
                        # even-padded so rows (2p, 2p+1) share rel.  The fp8
                        # one-hot row is 64 uint16 words with word (rel//2)
                        # equal to 0x0038 (rel even) or 0x3800 (rel odd) —
                        # fp8e4(1.0) in the right byte.  (ihalf == rel//2)
                        # * vsel computes it with all-16-bit tensor operands
                        # (fp8 writes from the DVE measure ~7x slower).
                        a_t = a_pool.tile([128, 64], u16, tag="onehot")
                        nc.vector.tensor_scalar(
                            a_t[:], iota_t[:, 0:64], relT[:, t:t + 1],
                            vselT[:, t:t + 1],
                            mybir.AluOpType.is_equal, mybir.AluOpType.mult)
                        oh_ap = a_t[:].bitcast(f8e4)
                    # the pair rows share one one-hot: broadcast it over the
                    # K-pair dim (stride-0 AP)
                    lhsT = oh_ap.unsqueeze(1).broadcast_to([128, 2, 128])

                    rhs = hid_t[:, (t - t0) * 2 * H:(t - t0 + 1) * 2 * H] \
                        .rearrange("p (two h) -> p two h", two=2)
                    if i == 0:
                        psum_cur = psum_s.tile([128, H], f32, tag="psum_s")
                    nc.tensor.matmul(
                        psum_cur[:], lhsT, rhs,
                        start=(i == 0), stop=(i == tb - 1), perf_mode=DR)
                    if FILL_START <= t < FILL_END:
                        for _ in range(FILL_N):
                            warm_fill()

                    if i == tb - 1:
                        sums_t = sums_pool.tile([128, H], f16, tag="sums")
                        nc.scalar.copy(sums_t[:], psum_cur[:])
                        pend_t.append((j, sums_t))
                        if len(pend_t) > 1:
                            stage_t(pend_t.pop(0))
                        if len(pend_c) > 1:
                            stage_c(pend_c.pop(0))
            while pend_t:
                stage_t(pend_t.pop(0))
            while pend_c:
                stage_c(pend_c.pop(0))
    nc.compile()
    return nc


def _quantize_ef(hidden, bag_edges):
    """fp8 E4M3 with per-(bag, h) error feedback down the rows."""
    starts = bag_edges[:-1]
    lens = np.diff(bag_edges)
    hq = np.zeros((N, H), F8E4)
    carry = np.zeros((NUM_BAGS, H), np.float32)
    for k in range(int(lens.max())):
        m = lens > k
        idx = starts[m] + k
        v = hidden[idx] + carry[m]
        q = v.astype(F8E4)
        hq[idx] = q
        carry[m] = v - q.astype(np.float32)
    return hq


def _pack_inputs(hidden, W, b, bag_id):
    counts = np.bincount(bag_id, minlength=NUM_BAGS)
    recip_all = (1.0 / np.maximum(counts, 1)).astype(np.float32)

    bag_edges = np.searchsorted(bag_id, np.arange(NUM_BAGS + 1))
    hq = _quantize_ef(hidden, bag_edges)

    lens = np.diff(bag_edges)                       # [8192]
    plens = lens + (lens & 1)                       # even-padded
    nblocks = NUM_BAGS // BLOCK_BAGS                # 64
    blk_plen = plens.reshape(nblocks, BLOCK_BAGS).sum(axis=1)
    tiles2 = np.maximum(1, -(-blk_plen // ROWS_DT))
    pos_tblks = tuple(
        int(x) for x in
        tiles2.reshape(NCORES, BLOCKS_PER_CORE).max(axis=0))
    T2 = sum(pos_tblks)
    offs2 = np.concatenate([[0], np.cumsum(pos_tblks)])

    Xp = np.zeros((NCORES, T2 * ROWS_DT, H), F8E4)
    rel2 = np.full((NCORES, T2 * 128), -1.0, dtype=np.float16)
    for bidx in range(nblocks):
        k, j = divmod(bidx, BLOCKS_PER_CORE)
        bl = lens[bidx * BLOCK_BAGS:(bidx + 1) * BLOCK_BAGS]
        pl = plens[bidx * BLOCK_BAGS:(bidx + 1) * BLOCK_BAGS]
        starts_dst = (offs2[j] * ROWS_DT +
                      np.concatenate([[0], np.cumsum(pl)[:-1]]))
        for bi in range(BLOCK_BAGS):
            Lb = int(bl[bi])
            d = int(starts_dst[bi])
            if Lb:
                s = int(bag_edges[bidx * BLOCK_BAGS + bi])
                Xp[k, d:d + Lb] = hq[s:s + Lb]
            PLb = int(pl[bi])
            if PLb:
                rel2[k, d // 2:(d + PLb) // 2] = bi

    wt_np = np.ascontiguousarray(W.T).astype(np.float16)      # [256, 128]
    wt_packed = np.concatenate([wt_np[0:128], wt_np[128:256]],
                               axis=1)                        # [128, 2C] f16
    b_np = np.tile(b.astype(np.float16), (128, 1))
    iota_np = np.tile((np.arange(128) % 64).astype(np.float16), (128, 1))

    chunks, coffs, cwidths = _chunk_layout(T2)
    in_maps = []
    for k in range(NCORES):
        hidc = (Xp[k].reshape(T2, 128, 2, H).transpose(1, 0, 2, 3)
                .reshape(128, T2 * 2 * H))
        rk = rel2[k].reshape(T2, 128)
        stream = np.zeros((128, coffs[-1]), F8E4)
        for c, (t0, t1) in enumerate(chunks):
            o0 = coffs[c]
            hw_ = (t1 - t0) * 2 * H
            stream[:, o0:o0 + hw_] = hidc[:, t0 * 2 * H:t1 * 2 * H]
            sl = 0
            for t in range(t0, t1):
                if _is_host(t):
                    rr = rk[t].astype(np.int32)
                    valid = rr >= 0
                    oh = np.zeros((128, 128), F8E4)
                    oh[np.arange(128)[valid], rr[valid]] = 1.0
                    o = o0 + hw_ + sl * 128
                    stream[:, o:o + 128] = oh
                    sl += 1
        relc = rel2[k].reshape(T2, 128).T.astype(np.float32)  # [128, T2]
        relh = np.where(relc >= 0, np.floor(relc / 2), -1.0)
        vsel = np.where(relc.astype(np.int32) % 2 == 0, 56.0, 14336.0)
        recc = recip_all[k * 1024:(k + 1) * 1024].reshape(
            BLOCKS_PER_CORE, 128).T.astype(np.float16)        # [128, 8]
        cst16_np = np.concatenate(
            [b_np, recc, iota_np], axis=1).astype(np.float16)
        cst_np = np.concatenate(
            [relh.astype(np.float32), vsel.astype(np.float32),
             np.ascontiguousarray(cst16_np).view(np.float32)], axis=1)
        in_maps.append({
            "hid": np.ascontiguousarray(stream),
            "cst": np.ascontiguousarray(cst_np),
            "wt": np.ascontiguousarray(wt_packed),
        })
    return in_maps, pos_tblks


def kernel(hidden, W, b, bag_id):
    global LAST_RESULTS
    hidden = np.asarray(hidden, dtype=np.float32)
    W = np.asarray(W, dtype=np.float32)
    b = np.asarray(b, dtype=np.float32)
    bag_id = np.asarray(bag_id).astype(np.int64)

    in_maps, pos_tblks = _pack_inputs(hidden, W, b, bag_id)

    key = (pos_tblks, CH2, FC2, MC2, HOSTMOD, HEAD_OH, WARMUP_MM, FILL_N,
           FILL_START, FILL_END, A_BUFS, H_BUFS)
    if key not in _prog_cache:
        _prog_cache[key] = _build_program(pos_tblks)
    nc = _prog_cache[key]

    trace = False
    if os.environ.get("BASS_TRACE"):
        trace = _install_ntff_shim()

    res = run_bass_kernel_spmd(nc, in_maps, core_ids=list(range(NCORES)),
                               trace=trace)
    LAST_RESULTS = res

    out = np.concatenate(
        [np.asarray(res.results[k]["out"]).astype(np.float32).reshape(1024, C)
         for k in range(NCORES)], axis=0)
    return out
